# revision 31
# baseline (speedup 1.0000x reference)
"""Depthwise causal Conv1d (k=4) + SiLU on 8 Trainium2 NeuronCores.

Problem: x [4, 4096, 2048] f32, w [2048, 4] f32,
out[b, t, d] = silu(sum_j w[d, j] * x[b, t - 3 + j, d])   (zero-padded left).

Sharding: 8 cores = 4 batches x 2 channel-halves. Depthwise conv is
independent per channel, so channel sharding needs no halo exchange.

Layout: each core receives its shard host-transposed to [channels, time]
(channels on SBUF partitions). The per-channel weight w[d, j] is then a
per-partition scalar (DVE tensor_scalar, ACT activation scale), and the
causal time shifts are free-dim AP offsets into one loaded tile.

Precision: x is host-cast to fp16 (halves input HBM traffic and enables
the DVE 4x tensor_scalar mode); products and the in-place add tree stay
fp16; SiLU computes fp32-internally on ACT and stores fp32. End-to-end
relative error ~6e-4.

Balance: per 128-channel block the 4 products are split between DVE
(tensor_scalar: 4x on 4B-aligned shifts 0/2, 2x on odd shifts) and ACT
(activation Copy with AP scale, 1x); 3 adds ride DVE, SiLU rides ACT.
3 of 8 blocks give DVE a third product so DVE and ACT land ~equal.
Loads issue on SyncE (HWDGE), stores on GpSimd (SWDGE) so a store
blocked on its SiLU never head-of-line-blocks load issue.
"""

import numpy as np

import concourse.bass as bass
import concourse.bacc as bacc
import concourse.mybir as mybir
from concourse.tile import TileContext
from concourse.bass_utils import run_bass_kernel_spmd

B, L, D = 4, 4096, 2048
K = 4
PAD = K - 1
N_CORES = 8
DH = D // 2            # channels per core
NBLK = DH // 128       # 128-partition channel blocks per core
ROWW = 4128            # DRAM row stride (fp16 elems): 64B-aligned rows

MID_DT = mybir.dt.float16
# blocks computed on the TensorEngine (diag(w) matmuls accumulating in PSUM)
PE_BLKS = {2, 4, 6}
# products per elementwise block that ride ACT (engine balance); rest on DVE
ACT_PRODUCTS = {0: 2, 1: 1, 3: 0, 5: 0, 7: 2}

_cache = {}


def _build_bass():
    nc = bacc.Bacc()
    xt = nc.dram_tensor("xt", [DH, ROWW], MID_DT, kind="ExternalInput")
    wt = nc.dram_tensor("wt", [128, NBLK * K], mybir.dt.float32, kind="ExternalInput")
    # diag(w) blocks for the PE path: partition p, col (blk*K + j)*128 + m
    # holds w[blk*128+p, j] iff m == p else 0
    wd = nc.dram_tensor("wd", [128, NBLK * K * 128], MID_DT, kind="ExternalInput")
    ot = nc.dram_tensor("ot", [DH, L], MID_DT, kind="ExternalOutput")
    f32 = mybir.dt.float32

    with TileContext(nc) as tc:
        with tc.tile_pool(name="pool", bufs=2) as pool, \
             tc.tile_pool(name="psum", bufs=2, space="PSUM") as psum_pool:
            # Warmup: a tiny Silu forces the silu activation-table set to
            # load during the initial DMA wait; Copy (used by the product
            # muls) is present in every set, so this is the only table
            # load in the whole kernel.
            warm = pool.tile([128, 2], MID_DT, tag="warm", bufs=1)
            nc.vector.memset(warm[:], 0.0)
            nc.scalar.activation(warm[:], warm[:], mybir.ActivationFunctionType.Silu)


            w = pool.tile([128, NBLK * K], f32, tag="w", bufs=1)
            nc.sync.dma_start(out=w[:], in_=wt[:, :])
            wdt = pool.tile([128, NBLK * K * 128], MID_DT, tag="wd", bufs=1)
            wd_loaded = False
            # Split first and last channel blocks into two time-chunks so
            # the pipeline ramps up and drains faster. PE blocks are pulled
            # one slot earlier so TensorE starts sooner.
            for blk in [0, 2, 1, 3, 4, 5, 6, 7]:
                r0 = blk * 128
                wj = lambda j: w[:, blk * K + j : blk * K + j + 1]
                if blk in PE_BLKS:
                    # TensorEngine path: load the full block, then per
                    # 2048-col PSUM chunk accumulate the 4 diag(w_j)
                    # matmuls (shift = free-dim offset on the moving
                    # operand), SiLU straight out of PSUM.
                    if not wd_loaded:
                        # deferred so it doesn't delay the first x loads
                        nc.sync.dma_start(out=wdt[:], in_=wd[:, :])
                        wd_loaded = True
                    x = pool.tile([128, L + PAD + 1], MID_DT, tag="x", bufs=4)
                    nc.sync.dma_start(
                        out=x[:, 0 : L + PAD], in_=xt[r0 : r0 + 128, 0 : L + PAD]
                    )
                    for half in range(2):
                        h0 = half * (L // 2)
                        ps = psum_pool.tile([128, L // 2], f32, tag="ps")
                        for j in range(K):
                            lw = wdt[:, (blk * K + j) * 128 : (blk * K + j + 1) * 128]
                            for c in range(L // 2 // 512):
                                nc.tensor.matmul(
                                    ps[:, c * 512 : (c + 1) * 512],
                                    lw,
                                    x[:, h0 + c * 512 + j : h0 + c * 512 + j + 512],
                                    start=(j == 0),
                                    stop=(j == K - 1),
                                )
                        o = pool.tile([128, L // 2], MID_DT, tag="o", bufs=4)
                        nc.scalar.activation(
                            o[:], ps[:], mybir.ActivationFunctionType.Silu
                        )
                        nc.gpsimd.dma_start(
                            out=ot[r0 : r0 + 128, h0 : h0 + L // 2], in_=o[:]
                        )
                    continue
                if blk == 0 or blk == NBLK - 1:
                    chunks = [(0, L // 2), (L // 2, L)]
                else:
                    chunks = [(0, L)]
                for t0, t1 in chunks:
                    tl = t1 - t0
                    x = pool.tile([128, tl + PAD + 1], MID_DT, tag="x", bufs=4)
                    nc.sync.dma_start(
                        out=x[:, 0 : tl + PAD], in_=xt[r0 : r0 + 128, t0 : t1 + PAD]
                    )

                    # qe holds the even-shift products [q0 | q2], qo the odd
                    # [q1 | q3], each as one contiguous [128, 2, tl] tile so
                    # both pair-adds run as a single tensor_tensor op.
                    qe = pool.tile([128, 2, tl], MID_DT, tag="qe", bufs=3)
                    qo = pool.tile([128, 2, tl], MID_DT, tag="qo", bufs=3)
                    # products, shift-rebased: q_j[:, t] = w_j * x[:, t + j]
                    n_act = ACT_PRODUCTS[blk]
                    nc.vector.tensor_scalar_mul(qe[:, 0, :], x[:, 0:tl], wj(0))
                    if n_act >= 1:
                        nc.scalar.mul(qo[:, 0, :], x[:, 1 : 1 + tl], wj(1))
                    else:
                        nc.vector.tensor_scalar_mul(qo[:, 0, :], x[:, 1 : 1 + tl], wj(1))
                    nc.vector.tensor_scalar_mul(qe[:, 1, :], x[:, 2 : 2 + tl], wj(2))
                    if n_act >= 2:
                        nc.scalar.mul(qo[:, 1, :], x[:, 3 : 3 + tl], wj(3))
                    else:
                        nc.vector.tensor_scalar_mul(qo[:, 1, :], x[:, 3 : 3 + tl], wj(3))

                    # both pair-adds in one op, then the final add in place
                    nc.vector.tensor_add(qe[:, :, :], qe[:, :, :], qo[:, :, :])
                    nc.vector.tensor_add(qe[:, 0, :], qe[:, 0, :], qe[:, 1, :])

                    o = pool.tile([128, tl], MID_DT, tag="o", bufs=4)
                    nc.scalar.activation(
                        o[:], qe[:, 0, :], mybir.ActivationFunctionType.Silu
                    )
                    nc.gpsimd.dma_start(out=ot[r0 : r0 + 128, t0:t1], in_=o[:])
    nc.compile()
    return nc


def _shard_inputs(x, w):
    in_maps = []
    for core in range(N_CORES):
        b, half = divmod(core, 2)
        d0 = half * DH
        xt = np.zeros((DH, ROWW), dtype=np.float16)
        xt[:, PAD : PAD + L] = x[b, :, d0 : d0 + DH].T.astype(np.float16)
        # w rows for this shard, rearranged so partition p holds the K
        # weights of channel blk*128 + p at free cols [blk*K, blk*K + K)
        w_sh = w[d0 : d0 + DH].reshape(NBLK, 128, K)
        wt = (
            w_sh.transpose(1, 0, 2).reshape(128, NBLK * K).astype(np.float32)
        )
        # diag blocks for the PE path
        wdv = np.zeros((128, NBLK, K, 128), dtype=np.float16)
        idx = np.arange(128)
        wdv[idx, :, :, idx] = w_sh.transpose(1, 0, 2).astype(np.float16)
        in_maps.append(
            {
                "xt": np.ascontiguousarray(xt),
                "wt": np.ascontiguousarray(wt),
                "wd": np.ascontiguousarray(wdv.reshape(128, NBLK * K * 128)),
            }
        )
    return in_maps


def kernel(x, w):
    x = np.asarray(x, dtype=np.float32)
    w = np.asarray(w, dtype=np.float32)
    assert x.shape == (B, L, D) and w.shape == (D, K)

    if "nc" not in _cache:
        _cache["nc"] = _build_bass()
    nc = _cache["nc"]

    res = run_bass_kernel_spmd(nc, _shard_inputs(x, w), core_ids=list(range(N_CORES)))
    _cache["last_results"] = res

    out = np.empty((B, L, D), dtype=np.float32)
    for core in range(N_CORES):
        b, half = divmod(core, 2)
        d0 = half * DH
        out[b, :, d0 : d0 + DH] = res.results[core]["ot"].T.astype(np.float32)
    return out


# revision 32
# speedup vs baseline: 1.0022x; 1.0022x over previous
"""Depthwise causal Conv1d (k=4) + SiLU on 8 Trainium2 NeuronCores.

Problem: x [4, 4096, 2048] f32, w [2048, 4] f32,
out[b, t, d] = silu(sum_j w[d, j] * x[b, t - 3 + j, d])   (zero-padded left).

Sharding: 8 cores = 4 batches x 2 channel-halves. Depthwise conv is
independent per channel, so channel sharding needs no halo exchange.

Layout: each core receives its shard host-transposed to [channels, time]
(channels on SBUF partitions). The per-channel weight w[d, j] is then a
per-partition scalar (DVE tensor_scalar, ACT activation scale), and the
causal time shifts are free-dim AP offsets into one loaded tile.

Precision: x is host-cast to fp16 (halves input HBM traffic and enables
the DVE 4x tensor_scalar mode); products and the in-place add tree stay
fp16; SiLU computes fp32-internally on ACT and stores fp32. End-to-end
relative error ~6e-4.

Balance: per 128-channel block the 4 products are split between DVE
(tensor_scalar: 4x on 4B-aligned shifts 0/2, 2x on odd shifts) and ACT
(activation Copy with AP scale, 1x); 3 adds ride DVE, SiLU rides ACT.
3 of 8 blocks give DVE a third product so DVE and ACT land ~equal.
Loads issue on SyncE (HWDGE), stores on GpSimd (SWDGE) so a store
blocked on its SiLU never head-of-line-blocks load issue.
"""

import numpy as np

import concourse.bass as bass
import concourse.bacc as bacc
import concourse.mybir as mybir
from concourse.tile import TileContext
from concourse.bass_utils import run_bass_kernel_spmd

B, L, D = 4, 4096, 2048
K = 4
PAD = K - 1
N_CORES = 8
DH = D // 2            # channels per core
NBLK = DH // 128       # 128-partition channel blocks per core
ROWW = 4128            # DRAM row stride (fp16 elems): 64B-aligned rows

MID_DT = mybir.dt.float16
# blocks computed on the TensorEngine (diag(w) matmuls accumulating in PSUM)
PE_BLKS = {2, 4, 6}
# products per elementwise block that ride ACT (engine balance); rest on DVE
ACT_PRODUCTS = {0: 2, 1: 1, 3: 0, 5: 0, 7: 2}

_cache = {}


def _build_bass():
    nc = bacc.Bacc()
    xt = nc.dram_tensor("xt", [DH, ROWW], MID_DT, kind="ExternalInput")
    wt = nc.dram_tensor("wt", [128, NBLK * K], mybir.dt.float32, kind="ExternalInput")
    # diag(w) blocks for the PE path: partition p, col (blk*K + j)*128 + m
    # holds w[blk*128+p, j] iff m == p else 0
    wd = nc.dram_tensor("wd", [128, NBLK * K * 128], MID_DT, kind="ExternalInput")
    ot = nc.dram_tensor("ot", [DH, L], MID_DT, kind="ExternalOutput")
    f32 = mybir.dt.float32

    with TileContext(nc) as tc:
        with tc.tile_pool(name="pool", bufs=2) as pool, \
             tc.tile_pool(name="psum", bufs=2, space="PSUM") as psum_pool:
            # Warmup: a tiny Silu forces the silu activation-table set to
            # load during the initial DMA wait; Copy (used by the product
            # muls) is present in every set, so this is the only table
            # load in the whole kernel.
            warm = pool.tile([128, 2], MID_DT, tag="warm", bufs=1)
            nc.vector.memset(warm[:], 0.0)
            nc.scalar.activation(warm[:], warm[:], mybir.ActivationFunctionType.Silu)


            w = pool.tile([128, NBLK * K], f32, tag="w", bufs=1)
            nc.sync.dma_start(out=w[:], in_=wt[:, :])
            wdt = pool.tile([128, NBLK * K * 128], MID_DT, tag="wd", bufs=1)
            wd_loaded = False
            # Split first and last channel blocks into two time-chunks so
            # the pipeline ramps up and drains faster. PE blocks are pulled
            # one slot earlier so TensorE starts sooner.
            for blk in [0, 2, 1, 4, 3, 6, 5, 7]:
                r0 = blk * 128
                wj = lambda j: w[:, blk * K + j : blk * K + j + 1]
                if blk in PE_BLKS:
                    # TensorEngine path: load the full block, then per
                    # 2048-col PSUM chunk accumulate the 4 diag(w_j)
                    # matmuls (shift = free-dim offset on the moving
                    # operand), SiLU straight out of PSUM.
                    if not wd_loaded:
                        # deferred so it doesn't delay the first x loads
                        nc.sync.dma_start(out=wdt[:], in_=wd[:, :])
                        wd_loaded = True
                    x = pool.tile([128, L + PAD + 1], MID_DT, tag="x", bufs=4)
                    nc.sync.dma_start(
                        out=x[:, 0 : L + PAD], in_=xt[r0 : r0 + 128, 0 : L + PAD]
                    )
                    for half in range(2):
                        h0 = half * (L // 2)
                        ps = psum_pool.tile([128, L // 2], f32, tag="ps")
                        for j in range(K):
                            lw = wdt[:, (blk * K + j) * 128 : (blk * K + j + 1) * 128]
                            for c in range(L // 2 // 512):
                                nc.tensor.matmul(
                                    ps[:, c * 512 : (c + 1) * 512],
                                    lw,
                                    x[:, h0 + c * 512 + j : h0 + c * 512 + j + 512],
                                    start=(j == 0),
                                    stop=(j == K - 1),
                                )
                        o = pool.tile([128, L // 2], MID_DT, tag="o", bufs=4)
                        nc.scalar.activation(
                            o[:], ps[:], mybir.ActivationFunctionType.Silu
                        )
                        nc.gpsimd.dma_start(
                            out=ot[r0 : r0 + 128, h0 : h0 + L // 2], in_=o[:]
                        )
                    continue
                if blk == 0 or blk == NBLK - 1:
                    chunks = [(0, L // 2), (L // 2, L)]
                else:
                    chunks = [(0, L)]
                for t0, t1 in chunks:
                    tl = t1 - t0
                    x = pool.tile([128, tl + PAD + 1], MID_DT, tag="x", bufs=4)
                    nc.sync.dma_start(
                        out=x[:, 0 : tl + PAD], in_=xt[r0 : r0 + 128, t0 : t1 + PAD]
                    )

                    # qe holds the even-shift products [q0 | q2], qo the odd
                    # [q1 | q3], each as one contiguous [128, 2, tl] tile so
                    # both pair-adds run as a single tensor_tensor op.
                    qe = pool.tile([128, 2, tl], MID_DT, tag="qe", bufs=3)
                    qo = pool.tile([128, 2, tl], MID_DT, tag="qo", bufs=3)
                    # products, shift-rebased: q_j[:, t] = w_j * x[:, t + j]
                    n_act = ACT_PRODUCTS[blk]
                    nc.vector.tensor_scalar_mul(qe[:, 0, :], x[:, 0:tl], wj(0))
                    if n_act >= 1:
                        nc.scalar.mul(qo[:, 0, :], x[:, 1 : 1 + tl], wj(1))
                    else:
                        nc.vector.tensor_scalar_mul(qo[:, 0, :], x[:, 1 : 1 + tl], wj(1))
                    nc.vector.tensor_scalar_mul(qe[:, 1, :], x[:, 2 : 2 + tl], wj(2))
                    if n_act >= 2:
                        nc.scalar.mul(qo[:, 1, :], x[:, 3 : 3 + tl], wj(3))
                    else:
                        nc.vector.tensor_scalar_mul(qo[:, 1, :], x[:, 3 : 3 + tl], wj(3))

                    # both pair-adds in one op, then the final add in place
                    nc.vector.tensor_add(qe[:, :, :], qe[:, :, :], qo[:, :, :])
                    nc.vector.tensor_add(qe[:, 0, :], qe[:, 0, :], qe[:, 1, :])

                    o = pool.tile([128, tl], MID_DT, tag="o", bufs=4)
                    nc.scalar.activation(
                        o[:], qe[:, 0, :], mybir.ActivationFunctionType.Silu
                    )
                    nc.gpsimd.dma_start(out=ot[r0 : r0 + 128, t0:t1], in_=o[:])
    nc.compile()
    return nc


def _shard_inputs(x, w):
    in_maps = []
    for core in range(N_CORES):
        b, half = divmod(core, 2)
        d0 = half * DH
        xt = np.zeros((DH, ROWW), dtype=np.float16)
        xt[:, PAD : PAD + L] = x[b, :, d0 : d0 + DH].T.astype(np.float16)
        # w rows for this shard, rearranged so partition p holds the K
        # weights of channel blk*128 + p at free cols [blk*K, blk*K + K)
        w_sh = w[d0 : d0 + DH].reshape(NBLK, 128, K)
        wt = (
            w_sh.transpose(1, 0, 2).reshape(128, NBLK * K).astype(np.float32)
        )
        # diag blocks for the PE path
        wdv = np.zeros((128, NBLK, K, 128), dtype=np.float16)
        idx = np.arange(128)
        wdv[idx, :, :, idx] = w_sh.transpose(1, 0, 2).astype(np.float16)
        in_maps.append(
            {
                "xt": np.ascontiguousarray(xt),
                "wt": np.ascontiguousarray(wt),
                "wd": np.ascontiguousarray(wdv.reshape(128, NBLK * K * 128)),
            }
        )
    return in_maps


def kernel(x, w):
    x = np.asarray(x, dtype=np.float32)
    w = np.asarray(w, dtype=np.float32)
    assert x.shape == (B, L, D) and w.shape == (D, K)

    if "nc" not in _cache:
        _cache["nc"] = _build_bass()
    nc = _cache["nc"]

    res = run_bass_kernel_spmd(nc, _shard_inputs(x, w), core_ids=list(range(N_CORES)))
    _cache["last_results"] = res

    out = np.empty((B, L, D), dtype=np.float32)
    for core in range(N_CORES):
        b, half = divmod(core, 2)
        d0 = half * DH
        out[b, :, d0 : d0 + DH] = res.results[core]["ot"].T.astype(np.float32)
    return out


# revision 33
# speedup vs baseline: 1.0186x; 1.0163x over previous
"""Depthwise causal Conv1d (k=4) + SiLU on 8 Trainium2 NeuronCores.

Problem: x [4, 4096, 2048] f32, w [2048, 4] f32,
out[b, t, d] = silu(sum_j w[d, j] * x[b, t - 3 + j, d])   (zero-padded left).

Sharding: 8 cores = 4 batches x 2 channel-halves. Depthwise conv is
independent per channel, so channel sharding needs no halo exchange.

Layout: each core receives its shard host-transposed to [channels, time]
(channels on SBUF partitions). The per-channel weight w[d, j] is then a
per-partition scalar (DVE tensor_scalar, ACT activation scale), and the
causal time shifts are free-dim AP offsets into one loaded tile.

Precision: x is host-cast to fp16 (halves input HBM traffic and enables
the DVE 4x tensor_scalar mode); products and the in-place add tree stay
fp16; SiLU computes fp32-internally on ACT and stores fp32. End-to-end
relative error ~6e-4.

Balance: per 128-channel block the 4 products are split between DVE
(tensor_scalar: 4x on 4B-aligned shifts 0/2, 2x on odd shifts) and ACT
(activation Copy with AP scale, 1x); 3 adds ride DVE, SiLU rides ACT.
3 of 8 blocks give DVE a third product so DVE and ACT land ~equal.
Loads issue on SyncE (HWDGE), stores on GpSimd (SWDGE) so a store
blocked on its SiLU never head-of-line-blocks load issue.
"""

import numpy as np

import concourse.bass as bass
import concourse.bacc as bacc
import concourse.mybir as mybir
from concourse.tile import TileContext
from concourse.bass_utils import run_bass_kernel_spmd

B, L, D = 4, 4096, 2048
K = 4
PAD = K - 1
N_CORES = 8
DH = D // 2            # channels per core
NBLK = DH // 128       # 128-partition channel blocks per core
ROWW = 4128            # DRAM row stride (fp16 elems): 64B-aligned rows

MID_DT = mybir.dt.float16
# blocks computed on the TensorEngine (diag(w) matmuls accumulating in PSUM)
PE_BLKS = {2, 4, 6}
# products per elementwise block that ride ACT (engine balance); rest on DVE
ACT_PRODUCTS = {0: 2, 1: 1, 3: 0, 5: 0, 7: 2}

_cache = {}


def _build_bass():
    nc = bacc.Bacc()
    xt = nc.dram_tensor("xt", [DH, ROWW], MID_DT, kind="ExternalInput")
    wt = nc.dram_tensor("wt", [128, NBLK * K], mybir.dt.float32, kind="ExternalInput")
    # diag(w) blocks for the PE path: partition p, col (blk*K + j)*128 + m
    # holds w[blk*128+p, j] iff m == p else 0
    wd = nc.dram_tensor("wd", [128, NBLK * K * 128], MID_DT, kind="ExternalInput")
    ot = nc.dram_tensor("ot", [DH, L], MID_DT, kind="ExternalOutput")
    f32 = mybir.dt.float32

    with TileContext(nc) as tc:
        with tc.tile_pool(name="pool", bufs=2) as pool, \
             tc.tile_pool(name="psum", bufs=2, space="PSUM") as psum_pool:
            # Warmup: a tiny Silu forces the silu activation-table set to
            # load during the initial DMA wait; Copy (used by the product
            # muls) is present in every set, so this is the only table
            # load in the whole kernel.
            warm = pool.tile([128, 2], MID_DT, tag="warm", bufs=1)
            nc.vector.memset(warm[:], 0.0)
            nc.scalar.activation(warm[:], warm[:], mybir.ActivationFunctionType.Silu)


            w = pool.tile([128, NBLK * K], f32, tag="w", bufs=1)
            nc.sync.dma_start(out=w[:], in_=wt[:, :])
            wdt = pool.tile([128, NBLK * K * 128], MID_DT, tag="wd", bufs=1)
            wd_loaded = False
            # Split first and last channel blocks into two time-chunks so
            # the pipeline ramps up and drains faster. PE blocks are pulled
            # one slot earlier so TensorE starts sooner.
            for blk in [0, 2, 1, 4, 3, 6, 5, 7]:
                r0 = blk * 128
                wj = lambda j: w[:, blk * K + j : blk * K + j + 1]
                if blk in PE_BLKS:
                    # TensorEngine path: load the full block, then per
                    # 2048-col PSUM chunk accumulate the 4 diag(w_j)
                    # matmuls (shift = free-dim offset on the moving
                    # operand), SiLU straight out of PSUM.
                    if not wd_loaded:
                        # deferred so it doesn't delay the first x loads
                        nc.sync.dma_start(out=wdt[:], in_=wd[:, :])
                        wd_loaded = True
                    x = pool.tile([128, L + PAD + 1], MID_DT, tag="x", bufs=5)
                    nc.sync.dma_start(
                        out=x[:, 0 : L + PAD], in_=xt[r0 : r0 + 128, 0 : L + PAD]
                    )
                    for half in range(2):
                        h0 = half * (L // 2)
                        ps = psum_pool.tile([128, L // 2], f32, tag="ps")
                        for j in range(K):
                            lw = wdt[:, (blk * K + j) * 128 : (blk * K + j + 1) * 128]
                            for c in range(L // 2 // 512):
                                nc.tensor.matmul(
                                    ps[:, c * 512 : (c + 1) * 512],
                                    lw,
                                    x[:, h0 + c * 512 + j : h0 + c * 512 + j + 512],
                                    start=(j == 0),
                                    stop=(j == K - 1),
                                )
                        o = pool.tile([128, L // 2], MID_DT, tag="o", bufs=5)
                        nc.scalar.activation(
                            o[:], ps[:], mybir.ActivationFunctionType.Silu
                        )
                        nc.gpsimd.dma_start(
                            out=ot[r0 : r0 + 128, h0 : h0 + L // 2], in_=o[:]
                        )
                    continue
                if blk == 0 or blk == NBLK - 1:
                    chunks = [(0, L // 2), (L // 2, L)]
                else:
                    chunks = [(0, L)]
                for t0, t1 in chunks:
                    tl = t1 - t0
                    x = pool.tile([128, tl + PAD + 1], MID_DT, tag="x", bufs=5)
                    nc.sync.dma_start(
                        out=x[:, 0 : tl + PAD], in_=xt[r0 : r0 + 128, t0 : t1 + PAD]
                    )

                    # qe holds the even-shift products [q0 | q2], qo the odd
                    # [q1 | q3], each as one contiguous [128, 2, tl] tile so
                    # both pair-adds run as a single tensor_tensor op.
                    qe = pool.tile([128, 2, tl], MID_DT, tag="qe", bufs=3)
                    qo = pool.tile([128, 2, tl], MID_DT, tag="qo", bufs=3)
                    # products, shift-rebased: q_j[:, t] = w_j * x[:, t + j]
                    n_act = ACT_PRODUCTS[blk]
                    nc.vector.tensor_scalar_mul(qe[:, 0, :], x[:, 0:tl], wj(0))
                    if n_act >= 1:
                        nc.scalar.mul(qo[:, 0, :], x[:, 1 : 1 + tl], wj(1))
                    else:
                        nc.vector.tensor_scalar_mul(qo[:, 0, :], x[:, 1 : 1 + tl], wj(1))
                    nc.vector.tensor_scalar_mul(qe[:, 1, :], x[:, 2 : 2 + tl], wj(2))
                    if n_act >= 2:
                        nc.scalar.mul(qo[:, 1, :], x[:, 3 : 3 + tl], wj(3))
                    else:
                        nc.vector.tensor_scalar_mul(qo[:, 1, :], x[:, 3 : 3 + tl], wj(3))

                    # both pair-adds in one op, then the final add in place
                    nc.vector.tensor_add(qe[:, :, :], qe[:, :, :], qo[:, :, :])
                    nc.vector.tensor_add(qe[:, 0, :], qe[:, 0, :], qe[:, 1, :])

                    o = pool.tile([128, tl], MID_DT, tag="o", bufs=5)
                    nc.scalar.activation(
                        o[:], qe[:, 0, :], mybir.ActivationFunctionType.Silu
                    )
                    nc.gpsimd.dma_start(out=ot[r0 : r0 + 128, t0:t1], in_=o[:])
    nc.compile()
    return nc


def _shard_inputs(x, w):
    in_maps = []
    for core in range(N_CORES):
        b, half = divmod(core, 2)
        d0 = half * DH
        xt = np.zeros((DH, ROWW), dtype=np.float16)
        xt[:, PAD : PAD + L] = x[b, :, d0 : d0 + DH].T.astype(np.float16)
        # w rows for this shard, rearranged so partition p holds the K
        # weights of channel blk*128 + p at free cols [blk*K, blk*K + K)
        w_sh = w[d0 : d0 + DH].reshape(NBLK, 128, K)
        wt = (
            w_sh.transpose(1, 0, 2).reshape(128, NBLK * K).astype(np.float32)
        )
        # diag blocks for the PE path
        wdv = np.zeros((128, NBLK, K, 128), dtype=np.float16)
        idx = np.arange(128)
        wdv[idx, :, :, idx] = w_sh.transpose(1, 0, 2).astype(np.float16)
        in_maps.append(
            {
                "xt": np.ascontiguousarray(xt),
                "wt": np.ascontiguousarray(wt),
                "wd": np.ascontiguousarray(wdv.reshape(128, NBLK * K * 128)),
            }
        )
    return in_maps


def kernel(x, w):
    x = np.asarray(x, dtype=np.float32)
    w = np.asarray(w, dtype=np.float32)
    assert x.shape == (B, L, D) and w.shape == (D, K)

    if "nc" not in _cache:
        _cache["nc"] = _build_bass()
    nc = _cache["nc"]

    res = run_bass_kernel_spmd(nc, _shard_inputs(x, w), core_ids=list(range(N_CORES)))
    _cache["last_results"] = res

    out = np.empty((B, L, D), dtype=np.float32)
    for core in range(N_CORES):
        b, half = divmod(core, 2)
        d0 = half * DH
        out[b, :, d0 : d0 + DH] = res.results[core]["ot"].T.astype(np.float32)
    return out
